# revision 33
# baseline (speedup 1.0000x reference)
"""AGaLiTe attention layer on 8 trn2 NeuronCores, data-parallel over B.

Self-contained: host-side numpy does layout prep (transposes, permutes,
sharding, cos-table); all heavy compute runs on-device via Bass/Tile.
"""
import math
import numpy as np

import concourse.bass as bass
import concourse.bacc as bacc
import concourse.mybir as mybir
import concourse.tile as tile
from concourse import library_config as _libcfg
from concourse.bass_utils import run_bass_kernel_spmd

T, B, IN, H, D, ETA, R = 128, 32, 512, 8, 64, 4, 4
DE = ETA * D
EPS = 1e-05
NCORES = 8
BL = B // NCORES          # 4 batches per core
BT = BL * T               # 512  (b,t) free size
NBT = ETA * BT            # 2048 (n,b,t) free size
PT = 4                    # channel ptiles of 128 over (h,d)=512
KT = IN // 128            # 4 contraction tiles
NO_KQV = 5 * H * D // 128  # 20 output tiles of kqv
OMEGAS = np.linspace(-math.pi, math.pi, R)  # [-pi, -pi/3, pi/3, pi]
# distinct |omega| values: w0 = pi/3 (r=1,2), w1 = pi (r=0,3)
W_OF_R = [1, 0, 0, 1]

F32 = mybir.dt.float32
FR = mybir.dt.float32r
F16 = mybir.dt.float16
AF = mybir.ActivationFunctionType
OP = mybir.AluOpType


def _np(x):
    return np.asarray(x)


def _consts():
    """Constant matrices embedded in the NEFF."""
    # REP[k, n]: [32, 128] maps small rows (h', n') -> tile-k partitions (h, d)
    # for slice n.  assembled [32, (k, n, 128)]
    repkn = np.zeros((32, PT, ETA, 128), np.float16)
    for k in range(PT):
        for n in range(ETA):
            for m in range(128):
                h = 2 * k + (1 if m >= 64 else 0)
                repkn[h * ETA + n, k, n, m] = 1.0
    repkn = repkn.reshape(32, PT * ETA * 128)
    # IND8[k]: [128, 8] indicator: col h gets partition p if h(k,p)==h
    ind8 = np.zeros((128, PT, 8), np.float16)
    for k in range(PT):
        for p in range(128):
            ind8[p, k, 2 * k + (1 if p >= 64 else 0)] = 1.0
    ind8 = ind8.reshape(128, PT * 8)
    # REP8[k]: [8, 128]: maps 8 h-rows -> tile-k partitions (h,d)
    rep8 = np.zeros((8, PT, 128), np.float16)
    for k in range(PT):
        for m in range(128):
            rep8[2 * k + (1 if m >= 64 else 0), k, m] = 1.0
    rep8 = rep8.reshape(8, PT * 128)
    return repkn, ind8, rep8


PROD16 = True  # fp16 products for the 4 kdq dots (norm dot stays f32)


def _build_program(use_nt):
    nc = bacc.Bacc("TRN2", target_bir_lowering=False, debug=False)

    def din(name, shape, dtype):
        return nc.dram_tensor(name, list(shape), dtype, kind="ExternalInput")

    x32 = din("x32", (KT, 128, BT), F32)        # inputs^T f32
    xr = din("xr", (KT, 128, BT), FR)           # same bits, f32r view
    wkp = din("wkp", (KT, 128, 3 * H * D), F32)  # keys/queries/gammas W^T
    wkv = din("wkv", (KT, 128, 2 * H * D), FR)   # values/beta W^T (f32r)
    wp = din("wp", (KT, 128, 3 * H * ETA), F32)
    wo = din("wo", (PT, 128, IN), F16)
    occ = din("occ", (1, 2 * BT), F32)
    kprev = din("kprev", (R, 512, ETA * BL), F32)
    vprev = din("vprev", (R, 512, BL), F16)
    sprev = din("sprev", (512, ETA * BL), F32)
    ntrow = din("ntrow", (1, BT), F32) if use_nt else None

    outT = nc.dram_tensor("outT", [KT, 128, BT], F32, kind="ExternalOutput")
    kf = nc.dram_tensor("kf", [R, 512, ETA * BL], F16, kind="ExternalOutput")
    vf = nc.dram_tensor("vf", [R, 512, BL], F16, kind="ExternalOutput")
    sf = nc.dram_tensor("sf", [512, ETA * BL], F32, kind="ExternalOutput")

    repkn_np, ind8_np, rep8_np = _consts()
    ind8_d = nc.inline_tensor(ind8_np.astype(np.float32), "ind8")
    ind8h_d = nc.inline_tensor(ind8_np.astype(np.float16), "ind8h")
    rep8_d = nc.inline_tensor(rep8_np.astype(np.float32), "rep8")

    with tile.TileContext(nc) as tc:
        _emit(tc, nc, locals(), use_nt)
    nc.compile()
    return nc


def _emit(tc, nc, tens, use_nt):
    x32, xr, wkp, wkv, wp, wo = (tens["x32"], tens["xr"], tens["wkp"],
                                 tens["wkv"], tens["wp"], tens["wo"])
    occ, kprev, vprev, sprev = (tens["occ"], tens["kprev"], tens["vprev"],
                                tens["sprev"])
    outT, kf, vf, sf = tens["outT"], tens["kf"], tens["vf"], tens["sf"]
    ind8_d, ind8h_d, rep8_d = tens["ind8_d"], tens["ind8h_d"], tens["rep8_d"]
    ntrow = tens.get("ntrow")
    ts = bass.ts

    with nc.allow_low_precision(reason="f32r/f16 value paths; norm path f32"), \
         tc.tile_pool(name="pp", bufs=1) as pp, \
         tc.tile_pool(name="dram", bufs=1, space="DRAM") as dpool, \
         tc.tile_pool(name="psum", bufs=1, space="PSUM") as psum:

        # ---------- persistent constants / small inputs ----------
        ind8 = pp.tile([128, PT * 8], F32, tag="ind8", name="ind8")
        ind8h = pp.tile([128, PT * 8], F16, tag="ind8h", name="ind8h")
        rep8 = pp.tile([8, PT * 128], FR, tag="rep8", name="rep8")
        nc.sync.dma_start(ind8[:], ind8_d.ap())
        nc.sync.dma_start(ind8h[:], ind8h_d.ap())
        nc.gpsimd.dma_start(rep8[:], rep8_d.ap())

        wo_sb = [pp.tile([128, IN], F16, tag=f"wo{k}", name=f"wo{k}")
                 for k in range(PT)]
        for k in range(PT):
            nc.sync.dma_start(wo_sb[k][:], wo.ap()[k])

        # occ/nt broadcast straight from DRAM (stride-0 partition source)
        occ_pb = pp.tile([128, 2 * BT], F32, tag="occpb", name="occpb")
        nc.sync.dma_start(occ_pb[:],
                          occ.ap().broadcast_to([128, 2 * BT]))
        if use_nt:
            nt_pb = pp.tile([128, BT], F32, tag="ntpb", name="ntpb")
            nc.sync.dma_start(nt_pb[:],
                              ntrow.ap().broadcast_to([128, BT]))

        # ---------- projection outputs (persistent) ----------
        rk = pp.tile([128, PT * BT], F32, tag="rk", name="rk")
        rq = pp.tile([128, PT * BT], F32, tag="rq", name="rq")
        sg = pp.tile([128, PT * BT], F32, tag="sg", name="sg")
        vv = pp.tile([128, PT * BT], F16, tag="vv", name="vv")
        sb_ = pp.tile([128, PT * BT], F16, tag="sb", name="sb")
        rp1 = pp.tile([32, BT], F32, tag="rp1", name="rp1")
        qn = pp.tile([32, BT], F32, tag="qn", name="qn")
        sp3 = pp.tile([32, BT], F32, tag="sp3", name="sp3")

        # ---------- projections (PE) in a scoped weight pool ----------
        with tc.tile_pool(name="wpool", bufs=1) as wpool:
            xT = [wpool.tile([128, BT], F32, tag=f"xT{i}", name=f"xT{i}")
                  for i in range(KT)]
            xTr = [wpool.tile([128, BT], FR, tag=f"xTr{i}", name=f"xTr{i}")
                   for i in range(KT)]
            for i in range(KT):
                nc.sync.dma_start(xT[i][:], x32.ap()[i])
                nc.sync.dma_start(xTr[i][:], xr.ap()[i])
            wkp_sb = [wpool.tile([128, 3 * H * D], F32, tag=f"wkp{i}",
                                 name=f"wkp{i}") for i in range(KT)]
            wkv_sb = [wpool.tile([128, 2 * H * D], FR, tag=f"wkv{i}",
                                 name=f"wkv{i}") for i in range(KT)]
            wp_sb = [wpool.tile([128, 3 * H * ETA], F32, tag=f"wp{i}",
                                name=f"wp{i}") for i in range(KT)]
            for i in range(KT):
                nc.sync.dma_start(wp_sb[i][:], wp.ap()[i])
            for blk in range(3):
                for i in range(KT):
                    nc.sync.dma_start(
                        wkp_sb[i][:, ts(blk, 512)],
                        wkp.ap()[i][:, ts(blk, 512)])
            for blk in range(2):
                for i in range(KT):
                    nc.sync.dma_start(
                        wkv_sb[i][:, ts(blk, 512)],
                        wkv.ap()[i][:, ts(blk, 512)])

            psp = psum.tile([96, BT], F32, tag="mm", name="pproj", bufs=3)
            for i in range(KT):
                nc.tensor.matmul(psp[:], wp_sb[i][:], xT[i][:],
                                 start=(i == 0), stop=(i == KT - 1))
            nc.vector.tensor_scalar_max(rp1[:], psp[0:32, :], 0.0)
            nc.vector.tensor_scalar_max(qn[:], psp[32:64, :], 0.0)
            nc.scalar.activation(sp3[:], psp[64:96, :], AF.Sigmoid)

            # precise half: i-order (keys, queries, gammas), k-major order
            for k_of in range(PT):
              for i_of in range(3):
                j = i_of * PT + k_of
                ps = psum.tile([128, BT], F32, tag="mm", name="kqv", bufs=3)
                for i in range(KT):
                    nc.tensor.matmul(ps[:], wkp_sb[i][:, ts(j, 128)],
                                     xT[i][:],
                                     start=(i == 0), stop=(i == KT - 1))
                dst = [rk, rq, sg][i_of][:, ts(k_of, BT)]
                if i_of in (0, 1):
                    nc.vector.tensor_scalar_max(dst, ps[:], 0.0)
                else:
                    nc.scalar.activation(dst, ps[:], AF.Sigmoid)
            # value half (f32r): i-order (values, beta)
            for k_of in range(PT):
              for i_of in range(2):
                j = i_of * PT + k_of
                ps = psum.tile([128, BT], F32, tag="mm", name="kqv", bufs=3)
                for i in range(KT):
                    nc.tensor.matmul(ps[:], wkv_sb[i][:, ts(j, 128)],
                                     xTr[i][:],
                                     start=(i == 0), stop=(i == KT - 1))
                dst = [vv, sb_][i_of][:, ts(k_of, BT)]
                if i_of == 0:
                    nc.vector.tensor_copy(dst, ps[:])
                else:
                    nc.scalar.activation(dst, ps[:], AF.Sigmoid)


        kprev_sb = pp.tile([128, R * PT * ETA * BL], F32, tag="kprev",
                           name="kprev")
        for r in range(R):
            for k in range(PT):
                nc.sync.dma_start(
                    kprev_sb[:, (r * PT + k) * ETA * BL:
                             (r * PT + k + 1) * ETA * BL],
                    kprev.ap()[r, ts(k, 128)])
        vprev_sb = pp.tile([128, R * PT * BL], F16, tag="vprev", name="vprev")
        for r in range(R):
            for k in range(PT):
                nc.sync.dma_start(
                    vprev_sb[:, (r * PT + k) * BL:(r * PT + k + 1) * BL],
                    vprev.ap()[r, ts(k, 128)])
        sprev_sb = pp.tile([128, PT * ETA * BL], F32, tag="sprev",
                           name="sprev")
        for k in range(PT):
            nc.sync.dma_start(
                sprev_sb[:, k * ETA * BL:(k + 1) * ETA * BL],
                sprev.ap()[ts(k, 128)])


        # ---------- small builds ----------
        kgd = rk
        vb = vv
        nc.vector.tensor_tensor(vb[:], vv[:], sb_[:], op=OP.mult)
        kgn = rp1
        nc.vector.tensor_tensor(kgn[:], rp1[:], sp3[:], op=OP.mult)

        # spill small mats sources to DRAM for replication reads
        scr = dpool.tile([96, BT], F32, tag="scr", name="scr")
        nc.sync.dma_start(scr[0:32, :], sp3[:])
        nc.sync.dma_start(scr[32:64, :], kgn[:])
        nc.sync.dma_start(scr[64:96, :], qn[:])

        def bcast_k(ap_bt, nrep=PT):
            return ap_bt[:, None, :].broadcast_to([ap_bt.shape[0], nrep, BT])

        def v4(ap):
            return ap.rearrange("p (k f) -> p k f", k=PT)

        kgd_w = [pp.tile([128, PT * BT], F16, tag=f"kgdw{w}", name=f"kgdw{w}")
                 for w in range(2)]
        xv_w = [pp.tile([128, PT * BT], F16, tag="xvw0", name="xvw0"), vb]
        nc.gpsimd.tensor_tensor(v4(xv_w[0][:]), v4(vb[:]),
                                bcast_k(occ_pb[:, ts(0, BT)]), op=OP.mult)
        nc.gpsimd.tensor_tensor(v4(xv_w[1][:]), v4(vb[:]),
                                bcast_k(occ_pb[:, ts(1, BT)]), op=OP.mult)

        db = sb_
        if use_nt:
            nc.vector.tensor_tensor(v4(sb_[:]), v4(sb_[:]), bcast_k(nt_pb[:]),
                                    op=OP.mult)
            nc.vector.tensor_tensor(v4(db[:]), bcast_k(nt_pb[:]), v4(sb_[:]),
                                    op=OP.subtract)
            sgnt = pp.tile([128, PT * BT], F32, tag="sgnt", name="sgnt")
            nc.vector.tensor_tensor(v4(sgnt[:]), v4(sg[:]), bcast_k(nt_pb[:]),
                                    op=OP.mult)
            sg_eff = sgnt
        else:
            nc.vector.tensor_scalar(db[:], sb_[:], -1.0, 1.0, op0=OP.mult,
                                    op1=OP.add)
            sg_eff = sg

        # ---------- V scans (fp16) ----------
        db0 = pp.tile([128, PT * BL], F16, tag="db0", name="db0")
        db_r = db[:].rearrange("p (kb t) -> p kb t", t=T)
        nc.gpsimd.tensor_copy(db0[:], db_r[:, :, 0])
        nc.gpsimd.memset(db_r[:, :, 0:1], 0.0)

        vout = [pp.tile([128, PT * BT], F16, tag=f"vout{r}", name=f"vout{r}")
                for r in range(R)]
        vx0 = pp.tile([128, 2 * PT * BL], F16, tag="vx0", name="vx0")
        for w in range(2):
            xr_ = xv_w[w][:].rearrange("p (kb t) -> p kb t", t=T)
            nc.gpsimd.tensor_copy(vx0[:, ts(w, PT * BL)], xr_[:, :, 0])
        vtmp = pp.tile([128, PT * BL], F16, tag="vtmp", name="vtmp")
        for r in [1, 2, 0, 3]:
            w = W_OF_R[r]
            xr_ = xv_w[w][:].rearrange("p (kb t) -> p kb t", t=T)
            vpr = vprev_sb[:, r * PT * BL:(r + 1) * PT * BL]
            nc.gpsimd.tensor_tensor(vtmp[:], db0[:], vpr, op=OP.mult)
            nc.gpsimd.tensor_tensor(xr_[:, :, 0], vtmp[:],
                                    vx0[:, ts(w, PT * BL)], op=OP.add)
            nc.vector.tensor_tensor_scan(vout[r][:], db[:], xv_w[w][:], 0.0,
                                         op0=OP.mult, op1=OP.add)
            vfin = vout[r][:].rearrange("p (k b t) -> p k b t", k=PT, b=BL)
            for k in range(PT):
                nc.sync.dma_start(vf.ap()[r, ts(k, 128)], vfin[:, k, :, T - 1])

        # ---------- per-ptile E pipeline ----------
        kdq_ps = psum.tile([8, (R + 1) * BT], F32, tag="kdq", name="kdq")

        def dot(prod_ap, k, col, ind):
            for n in range(ETA):
                nc.tensor.matmul(
                    kdq_ps[:, col * BT:(col + 1) * BT],
                    ind[:, ts(k, 8)],
                    prod_ap[:, ts(n, BT)],
                    start=(k == 0 and n == 0),
                    stop=(k == PT - 1 and n == ETA - 1),
                    skip_group_check=True)

        with tc.tile_pool(name="kk", bufs=1) as kkp:
            for k in range(PT):
                sl = slice(k * BT, (k + 1) * BT)
                nc.vector.tensor_tensor(kgd[:, sl], rk[:, sl], sg[:, sl],
                                        op=OP.mult)
                for w in range(2):
                    nc.vector.tensor_tensor(kgd_w[w][:, sl], kgd[:, sl],
                                            occ_pb[:, ts(w, BT)], op=OP.mult)

                def mk_mat(m_idx, tag):
                    mat = kkp.tile([128, NBT], F32, tag=tag, name=tag,
                                   bufs=2)
                    for hb in range(2):
                        h = 2 * k + hb
                        src = (scr[m_idx * 32:(m_idx + 1) * 32, :]
                               .rearrange("(h n) f -> h n f", n=ETA)[h]
                               [None, :, :].broadcast_to([64, ETA, BT]))
                        nc.sync.dma_start(
                            mat[hb * 64:(hb + 1) * 64, :]
                            .rearrange("p (n f) -> p n f", n=ETA), src)
                    return mat

                # dg = 1 - sg*sp
                spm = mk_mat(0, "spm")
                dg = kkp.tile([128, NBT], F32, tag="dg", name="dg", bufs=2)
                nc.vector.tensor_tensor(
                    dg[:].rearrange("p (n f) -> p n f", n=ETA),
                    spm[:].rearrange("p (n f) -> p n f", n=ETA),
                    bcast_k(sg_eff[:, sl], ETA), op=OP.mult)
                if use_nt:
                    nc.vector.tensor_tensor(
                        dg[:].rearrange("p (n f) -> p n f", n=ETA),
                        bcast_k(nt_pb[:], ETA),
                        dg[:].rearrange("p (n f) -> p n f", n=ETA),
                        op=OP.subtract)
                else:
                    nc.vector.tensor_scalar(dg[:], dg[:], -1.0, 1.0,
                                            op0=OP.mult, op1=OP.add)

                kgm = mk_mat(1, "kgm")
                xs = kkp.tile([128, NBT], F32, tag="xs", name="xs")
                xp3 = kkp.tile([128, NBT], F16, tag="xp3", name="xp3")
                xpi = kkp.tile([128, NBT], F16, tag="xpi", name="xpi")
                kga = kgm[:].rearrange("p (n f) -> p n f", n=ETA)
                nc.vector.tensor_tensor(
                    xs[:].rearrange("p (n f) -> p n f", n=ETA),
                    bcast_k(kgd[:, sl], ETA), kga, op=OP.mult)
                nc.vector.tensor_tensor(
                    xp3[:].rearrange("p (n f) -> p n f", n=ETA),
                    bcast_k(kgd_w[0][:, sl], ETA), kga, op=OP.mult)
                nc.vector.tensor_tensor(
                    xpi[:].rearrange("p (n f) -> p n f", n=ETA),
                    bcast_k(kgd_w[1][:, sl], ETA), kga, op=OP.mult)

                qnm = mk_mat(2, "qnm")
                qe = kkp.tile([128, NBT], F32, tag="qe", name="qe")
                nc.vector.tensor_tensor(
                    qe[:].rearrange("p (n f) -> p n f", n=ETA),
                    qnm[:].rearrange("p (n f) -> p n f", n=ETA),
                    bcast_k(rq[:, sl], ETA), op=OP.mult)
                qe16 = kkp.tile([128, NBT], F16, tag="qe16", name="qe16")
                nc.gpsimd.tensor_copy(qe16[:], qe[:])

                # boundary prep on dg
                dgr = dg[:].rearrange("p (nb t) -> p nb t", t=T)
                dg0 = kkp.tile([128, ETA * BL], F32, tag="dg0", name="dg0")
                nc.gpsimd.tensor_copy(dg0[:], dgr[:, :, 0])
                nc.gpsimd.memset(dgr[:, :, 0:1], 0.0)

                x0s = kkp.tile([128, 3 * ETA * BL], F32, tag="x0s",
                               name="x0s")
                for idx, xt in enumerate((xs, xp3, xpi)):
                    xtr = xt[:].rearrange("p (nb t) -> p nb t", t=T)
                    nc.gpsimd.tensor_copy(x0s[:, ts(idx, ETA * BL)],
                                          xtr[:, :, 0])
                ktmp = kkp.tile([128, ETA * BL], F32, tag="ktmp", name="ktmp")

                def fix_x0(xt, xsave_i, init_ap):
                    xtr = xt[:].rearrange("p (nb t) -> p nb t", t=T)
                    nc.gpsimd.tensor_tensor(ktmp[:], dg0[:], init_ap,
                                            op=OP.mult)
                    nc.gpsimd.tensor_tensor(xtr[:, :, 0], ktmp[:],
                                            x0s[:, ts(xsave_i, ETA * BL)],
                                            op=OP.add)

                def kpr(r):
                    return kprev_sb[:, (r * PT + k) * ETA * BL:
                                    (r * PT + k + 1) * ETA * BL]

                # S-scan + f32 norm dot (prod in-place into sout)
                fix_x0(xs, 0, sprev_sb[:, k * ETA * BL:(k + 1) * ETA * BL])
                sout = kkp.tile([128, NBT], F32, tag="sout", name="sout")
                nc.vector.tensor_tensor_scan(sout[:], dg[:], xs[:], 0.0,
                                             op0=OP.mult, op1=OP.add)
                nc.sync.dma_start(
                    sf.ap()[ts(k, 128)],
                    sout[:].rearrange("p (nb t) -> p nb t", t=T)[:, :, T - 1])
                nc.gpsimd.tensor_tensor(sout[:], sout[:], qe[:], op=OP.mult)
                dot(sout[:], k, R, ind8)

                # K-scans; r-pairs share x, serialize via re-fix
                pdt = F16 if PROD16 else F32
                pind = ind8h if PROD16 else ind8
                for xt, xsave_i, r_list in ((xp3, 1, (1, 2)),
                                            (xpi, 2, (0, 3))):
                    for r in r_list:
                        fix_x0(xt, xsave_i, kpr(r))
                        kout = kkp.tile([128, NBT], F16, tag="kout",
                                        name="kout", bufs=2)
                        nc.vector.tensor_tensor_scan(kout[:], dg[:], xt[:],
                                                     0.0, op0=OP.mult,
                                                     op1=OP.add)
                        nc.sync.dma_start(
                            kf.ap()[r, ts(k, 128)],
                            kout[:].rearrange("p (nb t) -> p nb t",
                                              t=T)[:, :, T - 1])
                        prod = kkp.tile([128, NBT], pdt, tag="prod",
                                        name="prod", bufs=2)
                        nc.gpsimd.tensor_tensor(prod[:], kout[:], qe16[:],
                                                op=OP.mult)
                        dot(prod[:], k, r, pind)

        # ---------- normalize + kv readout + out-proj ----------
        with tc.tile_pool(name="post", bufs=1) as post:
            kdq_sb = post.tile([8, (R + 1) * BT], FR, tag="kdqsb",
                               name="kdqsb")
            nc.scalar.copy(kdq_sb[:], kdq_ps[:])
            den = post.tile([8, BT], F32, tag="den", name="den")
            nc.vector.tensor_scalar(den[:], kdq_sb[:, R * BT:], 2.0 * R, EPS,
                                    op0=OP.mult, op1=OP.add)
            rden = post.tile([8, BT], F32, tag="rden", name="rden")
            nc.vector.reciprocal(rden[:], den[:])
            kdqd = kdq_sb
            nc.vector.tensor_tensor(
                kdqd[:, 0:R * BT].rearrange("p (r f) -> p r f", r=R),
                kdq_sb[:, 0:R * BT].rearrange("p (r f) -> p r f", r=R),
                rden[:][:, None, :].broadcast_to([8, R, BT]), op=OP.mult)

            attn = post.tile([128, PT * BT], F16, tag="attn", name="attn")
            for k in range(PT):
                kvms = []
                for r in range(R):
                    kvm_r = psum.tile([128, BT], F32, tag="mm", name="kvm",
                                      bufs=3)
                    nc.tensor.matmul(kvm_r[:], rep8[:, ts(k, 128)],
                                     kdqd[:, ts(r, BT)], start=True,
                                     stop=True, skip_group_check=True)
                    kvms.append(kvm_r)
                acc = post.tile([128, BT], F32, tag="kvacc", name="kvacc")
                t1 = post.tile([128, BT], F32, tag="kvt1", name="kvt1")
                nc.vector.tensor_tensor(acc[:], vout[0][:, ts(k, BT)],
                                        kvms[0][:], op=OP.mult)
                for r in range(1, R):
                    nc.vector.tensor_tensor(t1[:], vout[r][:, ts(k, BT)],
                                            kvms[r][:], op=OP.mult)
                    eng = nc.gpsimd if r % 2 == 0 else nc.vector
                    eng.tensor_tensor(acc[:], acc[:], t1[:], op=OP.add)
                nc.vector.tensor_copy(attn[:, ts(k, BT)], acc[:])

            for j in range(KT):
                po = psum.tile([128, BT], F32, tag="mm", name="oproj", bufs=3)
                for k in range(PT):
                    nc.tensor.matmul(po[:], wo_sb[k][:, ts(j, 128)],
                                     attn[:, ts(k, BT)],
                                     start=(k == 0), stop=(k == PT - 1))
                ot = post.tile([128, BT], F32, tag="otile", name="otile")
                nc.scalar.copy(ot[:], po[:])
                nc.sync.dma_start(outT.ap()[j], ot[:])


_PROG_CACHE = {}


def _get_program(use_nt):
    if use_nt not in _PROG_CACHE:
        _PROG_CACHE[use_nt] = _build_program(use_nt)
    return _PROG_CACHE[use_nt]


def _prepare(inputs, terminations, tilde_k_prev, tilde_v_prev, s_prev, tick,
             W_kqv, b_kqv, W_p, b_p, W_o, b_o):
    inputs = _np(inputs); terminations = _np(terminations)
    tilde_k_prev = _np(tilde_k_prev); tilde_v_prev = _np(tilde_v_prev)
    s_prev = _np(s_prev); tick = _np(tick)
    W_kqv = _np(W_kqv); b_kqv = _np(b_kqv); W_p = _np(W_p); b_p = _np(b_p)
    W_o = _np(W_o); b_o = _np(b_o)

    assert np.all(b_kqv == 0) and np.all(b_p == 0) and np.all(b_o == 0), \
        "nonzero biases not supported by this kernel build"
    use_nt = bool(np.any(terminations != 0))

    # ---- shared host prep (weights) ----
    # kqv rows (h, i, d) -> (i, h, d)
    wk_all = W_kqv.reshape(H, 5, D, IN)
    wkp_p = wk_all[:, [0, 1, 4]].transpose(1, 0, 2, 3).reshape(3 * H * D, IN)
    wkpT = np.ascontiguousarray(wkp_p.T.reshape(KT, 128, 3 * H * D)).astype(np.float32)
    wkv_p = wk_all[:, [2, 3]].transpose(1, 0, 2, 3).reshape(2 * H * D, IN)
    wkvT = np.ascontiguousarray(wkv_p.T.reshape(KT, 128, 2 * H * D)).astype(np.float32)
    # p rows (h, i, n) -> (i, h, n)
    wp_p = W_p.reshape(H, 3, ETA, IN).transpose(1, 0, 2, 3).reshape(96, IN)
    wpT = np.ascontiguousarray(wp_p.T.reshape(KT, 128, 96)).astype(np.float32)
    # Wo^T rows c=(h,d)
    woT = np.ascontiguousarray(W_o.T.reshape(PT, 128, IN)).astype(np.float16)

    nc = _get_program(use_nt)

    in_maps = []
    for c in range(NCORES):
        bs = slice(c * BL, (c + 1) * BL)
        x_bt = inputs[:, bs, :].transpose(1, 0, 2).reshape(BT, IN)
        x32 = np.ascontiguousarray(x_bt.T.reshape(KT, 128, BT)).astype(np.float32)
        # occ rows: w0=pi/3 (r=1), w1=pi (r=0); ticks = tick + t + 1
        tk = tick[bs, 0][:, None] + np.arange(1, T + 1, dtype=np.float32)[None, :]
        occ_c = np.stack([np.cos(tk * abs(OMEGAS[1])),
                          np.cos(tk * abs(OMEGAS[0]))], 0)  # [2, BL, T]
        occ_c = occ_c.reshape(1, 2 * BT).astype(np.float32)
        # kprev [b,r,h,de] -> [r, (h,d), (n,b)]
        kp = tilde_k_prev[bs].reshape(BL, R, H, D, ETA)
        kp = kp.transpose(1, 2, 3, 4, 0).reshape(R, 512, ETA * BL)
        vp = tilde_v_prev[bs].transpose(1, 2, 3, 0).reshape(R, 512, BL)
        sp = s_prev[bs].reshape(BL, H, D, ETA).transpose(1, 2, 3, 0)
        sp = sp.reshape(512, ETA * BL)
        im = {
            "x32": x32, "xr": x32, "wkp": wkpT, "wkv": wkvT, "wp": wpT,
            "wo": woT,
            "occ": occ_c,
            "kprev": np.ascontiguousarray(kp).astype(np.float32),
            "vprev": np.ascontiguousarray(vp).astype(np.float16),
            "sprev": np.ascontiguousarray(sp).astype(np.float32),
        }
        if use_nt:
            nt = (1.0 - terminations[:, bs]).T.reshape(1, BT)
            im["ntrow"] = np.ascontiguousarray(nt).astype(np.float32)
        in_maps.append(im)
    return nc, in_maps, tick


def _unshard(core_results, tick):
    out = np.empty((T, B, IN), np.float32)
    kf_o = np.empty((B, R, H, DE), np.float32)
    vf_o = np.empty((B, R, H, D), np.float32)
    sf_o = np.empty((B, H, DE), np.float32)
    ncores = len(core_results)
    for c in range(ncores):
        bs = slice(c * BL, (c + 1) * BL)
        r = core_results[c]
        out[:, bs, :] = (r["outT"].reshape(IN, BL, T)
                         .transpose(2, 1, 0).astype(np.float32))
        kf_o[bs] = (r["kf"].astype(np.float32)
                    .reshape(R, H, D, ETA, BL).transpose(4, 0, 1, 2, 3)
                    .reshape(BL, R, H, DE))
        vf_o[bs] = (r["vf"].astype(np.float32)
                    .reshape(R, H, D, BL).transpose(3, 0, 1, 2))
        sf_o[bs] = (r["sf"].astype(np.float32)
                    .reshape(H, D, ETA, BL).transpose(3, 0, 1, 2)
                    .reshape(BL, H, DE))
    tick_out = tick.astype(np.float32) + np.float32(T)
    return out[:, :ncores * BL], kf_o[:ncores * BL], vf_o[:ncores * BL], \
        sf_o[:ncores * BL], tick_out


def kernel(inputs, terminations, tilde_k_prev, tilde_v_prev, s_prev, tick,
           W_kqv, b_kqv, W_p, b_p, W_o, b_o, _trace=False):
    nc, in_maps, tick = _prepare(
        inputs, terminations, tilde_k_prev, tilde_v_prev, s_prev, tick,
        W_kqv, b_kqv, W_p, b_p, W_o, b_o)
    res = run_bass_kernel_spmd(nc, in_maps, list(range(NCORES)), trace=_trace)
    if _trace:
        kernel.last_results = res
    out, kf_o, vf_o, sf_o, tick_out = _unshard(res.results, _np(tick))
    return out, kf_o, vf_o, sf_o, tick_out
